# revision 28
# baseline (speedup 1.0000x reference)
"""Distributed Trainium2 kernel for AdaptiveSocialFusion (GNN message passing).

Row-parallel across 8 NeuronCores: each core owns B/8 = 1024 output rows.
Instead of an AllGather (entry barrier + serial collective measured ~100us on
this stack), the host replicates the shared operands to every core as inputs:
  - xT   [D, B]  bf16: nodes transposed (sim lhsT, raw)
  - na   [B, D+1] bf16: nodes plus a ones column (aggregation rhs; the ones
    column makes the aggregation matmul emit row_sums for free)
Each core computes the all-rows norm scales r_j = temp/max(||x_j||,eps) on
device (elementwise square + per-tile ones-matmul + short chain) and applies
them inside the sigmoid activation as a per-partition scale, so the sim
matmul runs on raw xT while staying exactly sigmoid((sim-thresh)*temp).
Local rows are normalized in f32 and transposed on PE for the rhs side.
Then a fused pipeline per simT tile [128 j, 512 i]: matmul -> sigmoid ->
patient mask (bf16 codes, not_equal * mult) -> aggregation matmul into PSUM
accumulators; FiLM MLP + gating per 512-row chunk. Output assembled on host.
"""
import numpy as np

B = 8192
D = 256
H = 256
M2 = 512          # 2*D
NCORES = 8
R = B // NCORES   # 1024 rows per core
NIT = R // 128    # 8 i-subtiles per core
NJT = B // 128    # 64 j-tiles
NIC = 2           # i-chunks of 512
IC = 512
PAD = 128         # band halo rows each side (max patient group << 128)
BND = R + 2 * PAD # 1280 band rows per core
NBJ = BND // 128  # 10 band j-tiles
NCB = 6           # correction band tiles per i-chunk


def _build(thresh: float, temp: float):
    import concourse.bass as bass
    import concourse.tile as tile
    from concourse import bacc, mybir, masks

    f32 = mybir.dt.float32
    bf16 = mybir.dt.bfloat16
    AF = mybir.ActivationFunctionType
    ALU = mybir.AluOpType
    AX = mybir.AxisListType

    nc = bacc.Bacc("TRN2", target_bir_lowering=False, debug=False, num_devices=NCORES)

    nodes = nc.declare_dram_parameter("nodes", [R, D], f32, isOutput=False)
    xT = nc.declare_dram_parameter("xT", [D, B], bf16, isOutput=False)
    na = nc.declare_dram_parameter("na", [B, D + 1], bf16, isOutput=False)
    xT_band = nc.declare_dram_parameter("xT_band", [D, BND], bf16, isOutput=False)
    na_band = nc.declare_dram_parameter("na_band", [BND, D + 1], bf16, isOutput=False)
    pa_band = nc.declare_dram_parameter("pa_band", [128, NBJ], f32, isOutput=False)
    p_bcast = nc.declare_dram_parameter("p_bcast", [128, R], bf16, isOutput=False)
    p_all = nc.declare_dram_parameter("p_all", [128, NJT], f32, isOutput=False)
    w1 = nc.declare_dram_parameter("w1", [D, H], f32, isOutput=False)
    w2 = nc.declare_dram_parameter("w2", [H, M2], f32, isOutput=False)
    b1c = nc.declare_dram_parameter("b1c", [128, H // 128], f32, isOutput=False)
    b2b = nc.declare_dram_parameter("b2b", [128, M2], f32, isOutput=False)
    out = nc.declare_dram_parameter("out", [R, D], f32, isOutput=True)

    with tile.TileContext(nc) as tc:
        with (
            tc.tile_pool(name="const", bufs=1) as cpool,
            tc.tile_pool(name="resident", bufs=1) as rpool,
            tc.tile_pool(name="rot", bufs=2) as rot,
            tc.tile_pool(name="small", bufs=2) as small,
            tc.tile_pool(name="simp", bufs=2, space="PSUM") as simp,
            tc.tile_pool(name="wnp", bufs=1, space="PSUM") as wnp,
            tc.tile_pool(name="tailp", bufs=1, space="PSUM") as tailp,
        ):
            ident = cpool.tile([128, 128], bf16, tag="ident", name="ident")
            masks.make_identity(nc, ident[:])
            ones_sb = cpool.tile([128, 1], bf16, tag="ones", name="ones")
            nc.vector.memset(ones_sb[:], 1.0)

            # ---- stream in the replicated operands (chunked; na on the gpsimd
            # queue so DMA issue cost is split across two sequencers), and
            # compute the all-rows norm scales rt[j] = temp/max(||x_j||,eps)
            # chunk-by-chunk so the first sigmoid doesn't wait on all of xT.
            NCI = 8
            CW = B // NCI           # 1024 xT columns (8 j-tiles) per chunk
            NAW = 4 * 264           # 4 padded j-tiles per na macro-tile
            xT_sb = [rpool.tile([128, B], bf16, tag=f"xT{dt}", name=f"xT{dt}")
                     for dt in range(2)]
            na4 = [rpool.tile([128, NAW], bf16, tag=f"na4_{q}", name=f"na4_{q}")
                   for q in range(16)]
            na_view = na[:, :].rearrange("(q u p) c -> q p u c", p=128, u=4)
            rt_sb = cpool.tile([128, NJT], f32, tag="rt", name="rt")
            s_ps = tailp.tile([128, NJT], f32, tag="mlp", name="s_ps")
            for ci in range(NCI):
                cs = slice(ci * CW, (ci + 1) * CW)
                for dt in range(2):
                    nc.sync.dma_start(xT_sb[dt][:, cs], xT[dt * 128:(dt + 1) * 128, cs])
                for q in (2 * ci, 2 * ci + 1):
                    dst = na4[q][:].rearrange("p (u c) -> p u c", c=264)
                    nc.sync.dma_start(dst[:, :, 0:D + 1], na_view[q])
                xsq = [rot.tile([128, CW], bf16, tag=f"xsq{dt}", name=f"xsq{dt}")
                       for dt in range(2)]
                for dt in range(2):
                    nc.vector.tensor_mul(xsq[dt][:], xT_sb[dt][:, cs], xT_sb[dt][:, cs])
                for u in range(CW // 128):
                    jt = ci * (CW // 128) + u
                    for dt in range(2):
                        nc.tensor.matmul(s_ps[:, jt:jt + 1],
                                         xsq[dt][:, u * 128:(u + 1) * 128],
                                         ones_sb[:],
                                         start=(dt == 0), stop=(dt == 1))
                js = slice(ci * (CW // 128), (ci + 1) * (CW // 128))
                y = small.tile([128, CW // 128], f32, tag="ynorm", name="ynorm")
                nc.scalar.activation(y[:], s_ps[:, js], AF.Sqrt)
                nc.vector.tensor_scalar_max(y[:], y[:], 1e-12)
                t1 = small.tile([128, CW // 128], f32, tag="t1n", name="t1n")
                nc.vector.reciprocal(t1[:], y[:])
                t2 = small.tile([128, CW // 128], f32, tag="t2n", name="t2n")
                nc.vector.tensor_mul(t2[:], s_ps[:, js], t1[:])
                nc.vector.tensor_add(y[:], y[:], t2[:])
                nc.vector.tensor_scalar_mul(y[:], y[:], 0.5)
                nc.vector.reciprocal(t1[:], y[:])
                nc.vector.tensor_scalar_mul(rt_sb[:, js], t1[:], temp)

            def na_rhs(jt):
                q, u = jt // 4, jt % 4
                return na4[q][:, u * 264:u * 264 + D + 1]

            # ---- own-band inputs for the same-patient correction pass
            xTb_sb = [rpool.tile([128, BND], bf16, tag=f"xTb{dt}", name=f"xTb{dt}")
                      for dt in range(2)]
            for dt in range(2):
                nc.sync.dma_start(xTb_sb[dt][:], xT_band[dt * 128:(dt + 1) * 128, :])
            nab_sb = rpool.tile([128, NBJ * 264], bf16, tag="nab", name="nab")
            nab_dst = nab_sb[:].rearrange("p (u c) -> p u c", c=264)
            nab_view = na_band[:, :].rearrange("(u p) c -> p u c", p=128)
            nc.sync.dma_start(nab_dst[:, :, 0:D + 1], nab_view)
            pab_sb = cpool.tile([128, NBJ], f32, tag="pab", name="pab")
            nc.sync.dma_start(pab_sb[:], pa_band[:, :])
            # band norm scales (bit-exact replica of the global rt pipeline)
            sb_ps = tailp.tile([128, NBJ], f32, tag="mlp", name="sb_ps")
            xsqb = [rot.tile([128, BND], bf16, tag=f"xsqb{dt}", name=f"xsqb{dt}")
                    for dt in range(2)]
            for dt in range(2):
                nc.vector.tensor_mul(xsqb[dt][:], xTb_sb[dt][:], xTb_sb[dt][:])
            for bj in range(NBJ):
                for dt in range(2):
                    nc.tensor.matmul(sb_ps[:, bj:bj + 1],
                                     xsqb[dt][:, bj * 128:(bj + 1) * 128],
                                     ones_sb[:],
                                     start=(dt == 0), stop=(dt == 1))
            yb = small.tile([128, NBJ], f32, tag="ynorm", name="ynorm")
            nc.scalar.activation(yb[:], sb_ps[:], AF.Sqrt)
            nc.vector.tensor_scalar_max(yb[:], yb[:], 1e-12)
            tb1 = small.tile([128, NBJ], f32, tag="t1n", name="t1n")
            nc.vector.reciprocal(tb1[:], yb[:])
            tb2 = small.tile([128, NBJ], f32, tag="t2n", name="t2n")
            nc.vector.tensor_mul(tb2[:], sb_ps[:], tb1[:])
            nc.vector.tensor_add(yb[:], yb[:], tb2[:])
            nc.vector.tensor_scalar_mul(yb[:], yb[:], 0.5)
            nc.vector.reciprocal(tb1[:], yb[:])
            rtb_sb = cpool.tile([128, NBJ], f32, tag="rtb", name="rtb")
            nc.vector.tensor_scalar_mul(rtb_sb[:], tb1[:], temp)

            # ---- local rows: f32 normalize + transpose -> fnT_loc [D, R]
            nodes_sb = []
            fnT_loc = [rpool.tile([128, R], bf16, tag=f"fnTloc{dt}", name=f"fnTloc{dt}")
                       for dt in range(2)]
            sloc = cpool.tile([128, NIT], f32, tag="sloc", name="sloc")
            for t in range(NIT):
                nt = rpool.tile([128, D], f32, tag=f"nodes{t}", name=f"nodes{t}")
                nc.sync.dma_start(nt[:], nodes[t * 128:(t + 1) * 128, :])
                nodes_sb.append(nt)
                sq = rot.tile([128, D], f32, tag="sq", name="sq")
                nc.vector.tensor_mul(sq[:], nt[:], nt[:])
                nc.vector.reduce_sum(sloc[:, t:t + 1], sq[:], axis=AX.X)
            yl = small.tile([128, NIT], f32, tag="yl", name="yl")
            nc.scalar.activation(yl[:], sloc[:], AF.Sqrt)
            nc.vector.tensor_scalar_max(yl[:], yl[:], 1e-12)
            tl1 = small.tile([128, NIT], f32, tag="tl1", name="tl1")
            nc.vector.reciprocal(tl1[:], yl[:])
            tl2 = small.tile([128, NIT], f32, tag="tl2", name="tl2")
            nc.vector.tensor_mul(tl2[:], sloc[:], tl1[:])
            nc.vector.tensor_add(yl[:], yl[:], tl2[:])
            nc.vector.tensor_scalar_mul(yl[:], yl[:], 0.5)
            rl = small.tile([128, NIT], f32, tag="rl", name="rl")
            nc.vector.reciprocal(rl[:], yl[:])
            for t in range(NIT):
                fn = rot.tile([128, D], bf16, tag="fn", name="fn")
                nc.vector.tensor_scalar_mul(fn[:], nodes_sb[t][:], rl[:, t:t + 1])
                for dt in range(2):
                    ps_t = tailp.tile([128, 128], bf16, tag="tp", name="tp")
                    nc.tensor.matmul(ps_t[:], fn[:, dt * 128:(dt + 1) * 128],
                                     ident[:], is_transpose=True)
                    nc.vector.tensor_copy(fnT_loc[dt][:, t * 128:(t + 1) * 128],
                                          ps_t[:])

            # ---- constants
            pb_sb = cpool.tile([128, R], bf16, tag="pb", name="pb")
            nc.sync.dma_start(pb_sb[:], p_bcast[:, :])
            pa_sb = cpool.tile([128, NJT], f32, tag="pa", name="pa")
            nc.sync.dma_start(pa_sb[:], p_all[:, :])
            w1_sb = []
            for dt in range(2):
                wt = cpool.tile([128, H], f32, tag=f"w1f{dt}", name=f"w1f{dt}")
                nc.sync.dma_start(wt[:], w1[dt * 128:(dt + 1) * 128, :])
                wb = cpool.tile([128, H], bf16, tag=f"w1b{dt}", name=f"w1b{dt}")
                nc.vector.tensor_copy(wb[:], wt[:])
                w1_sb.append(wb)
            w2_sb = []
            for kt in range(2):
                wt = cpool.tile([128, M2], f32, tag=f"w2f{kt}", name=f"w2f{kt}")
                nc.sync.dma_start(wt[:], w2[kt * 128:(kt + 1) * 128, :])
                wb = cpool.tile([128, M2], bf16, tag=f"w2b{kt}", name=f"w2b{kt}")
                nc.vector.tensor_copy(wb[:], wt[:])
                w2_sb.append(wb)
            b1_sb = cpool.tile([128, H // 128], f32, tag="b1", name="b1")
            nc.sync.dma_start(b1_sb[:], b1c[:, :])
            b2_sb = cpool.tile([128, M2], f32, tag="b2", name="b2")
            nc.sync.dma_start(b2_sb[:], b2b[:, :])
            nbias_sb = cpool.tile([128, 1], f32, tag="nbias", name="nbias")
            nc.vector.memset(nbias_sb[:], -thresh * temp)

            # ---- main fused loop
            for ic in range(NIC):
                wn_ps = [wnp.tile([128, D + 1], f32, tag=f"wn{m}", name=f"wn{m}")
                         for m in range(4)]
                # same-patient correction: subtract masked pairs from the
                # (otherwise unmasked) aggregation using the own-band inputs.
                # adj values here are bit-exact replicas of the main loop's,
                # so the PSUM cancellation is exact to f32 rounding.
                for ci_, bj in enumerate(range(4 * ic, 4 * ic + NCB)):
                    sim_ps = simp.tile([128, IC], f32, tag="sim", name="sim")
                    nc.tensor.matmul(sim_ps[:],
                                     xTb_sb[0][:, bj * 128:(bj + 1) * 128],
                                     fnT_loc[0][:, ic * IC:(ic + 1) * IC],
                                     start=True, stop=False)
                    nc.tensor.matmul(sim_ps[:],
                                     xTb_sb[1][:, bj * 128:(bj + 1) * 128],
                                     fnT_loc[1][:, ic * IC:(ic + 1) * IC],
                                     start=False, stop=True)
                    adjT = rot.tile([128, IC], bf16, tag="adj", name="adj")
                    nc.scalar.activation(adjT[:], sim_ps[:], AF.Sigmoid,
                                         bias=nbias_sb[:],
                                         scale=rtb_sb[:, bj:bj + 1])
                    eqn = rot.tile([128, IC], bf16, tag="neq", name="neq")
                    nc.vector.tensor_scalar(eqn[:], pb_sb[:, ic * IC:(ic + 1) * IC],
                                            pab_sb[:, bj:bj + 1], -1.0,
                                            op0=ALU.is_equal, op1=ALU.mult)
                    nc.vector.tensor_mul(adjT[:], adjT[:], eqn[:])
                    for m in range(4):
                        nc.tensor.matmul(wn_ps[m][:],
                                         adjT[:, m * 128:(m + 1) * 128],
                                         nab_sb[:, bj * 264:bj * 264 + D + 1],
                                         start=(ci_ == 0), stop=False)
                for jt in range(NJT):
                    sim_ps = simp.tile([128, IC], f32, tag="sim", name="sim")
                    nc.tensor.matmul(sim_ps[:],
                                     xT_sb[0][:, jt * 128:(jt + 1) * 128],
                                     fnT_loc[0][:, ic * IC:(ic + 1) * IC],
                                     start=True, stop=False)
                    nc.tensor.matmul(sim_ps[:],
                                     xT_sb[1][:, jt * 128:(jt + 1) * 128],
                                     fnT_loc[1][:, ic * IC:(ic + 1) * IC],
                                     start=False, stop=True)
                    adjT = rot.tile([128, IC], bf16, tag="adj", name="adj")
                    nc.scalar.activation(adjT[:], sim_ps[:], AF.Sigmoid,
                                         bias=nbias_sb[:],
                                         scale=rt_sb[:, jt:jt + 1])
                    for m in range(4):
                        nc.tensor.matmul(wn_ps[m][:],
                                         adjT[:, m * 128:(m + 1) * 128],
                                         na_rhs(jt),
                                         start=False, stop=(jt == NJT - 1))

                # ---- per-chunk tail: row normalize, FiLM MLP, combine
                gates, wn_sb = [], []
                for m in range(4):
                    rs = small.tile([128, 1], f32, tag=f"rs{m}", name=f"rs{m}")
                    nc.vector.tensor_scalar_add(rs[:], wn_ps[m][:, D:D + 1], 1e-6)
                    gate = small.tile([128, 1], f32, tag=f"gate{m}", name=f"gate{m}")
                    nc.scalar.activation(gate[:], rs[:], AF.Tanh)
                    gates.append(gate)
                    rcp = small.tile([128, 1], f32, tag=f"rcp{m}", name=f"rcp{m}")
                    nc.vector.reciprocal(rcp[:], rs[:])
                    wnb = rot.tile([128, D], bf16, tag=f"wnsb{m}", name=f"wnsb{m}")
                    nc.vector.tensor_scalar_mul(wnb[:], wn_ps[m][:, 0:D], rcp[:])
                    wn_sb.append(wnb)

                wnT = [rot.tile([128, IC], bf16, tag=f"wnT{dt}", name=f"wnT{dt}")
                       for dt in range(2)]
                for m in range(4):
                    for dt in range(2):
                        ps_t = tailp.tile([128, 128], bf16, tag="tp", name="tp")
                        nc.tensor.matmul(ps_t[:], wn_sb[m][:, dt * 128:(dt + 1) * 128],
                                         ident[:], is_transpose=True)
                        nc.vector.tensor_copy(wnT[dt][:, m * 128:(m + 1) * 128],
                                              ps_t[:])

                hT = []
                for kt in range(2):
                    h_ps = tailp.tile([128, IC], f32, tag="mlp", name="mlp")
                    nc.tensor.matmul(h_ps[:], w1_sb[0][:, kt * 128:(kt + 1) * 128],
                                     wnT[0][:], start=True, stop=False)
                    nc.tensor.matmul(h_ps[:], w1_sb[1][:, kt * 128:(kt + 1) * 128],
                                     wnT[1][:], start=False, stop=True)
                    ht = rot.tile([128, IC], bf16, tag=f"hT{kt}", name=f"hT{kt}")
                    nc.scalar.activation(ht[:], h_ps[:], AF.Relu,
                                         bias=b1_sb[:, kt:kt + 1])
                    hT.append(ht)

                for m in range(4):
                    it = ic * 4 + m
                    f_ps = tailp.tile([128, M2], f32, tag="mlp", name="mlp")
                    nc.tensor.matmul(f_ps[:], hT[0][:, m * 128:(m + 1) * 128],
                                     w2_sb[0][:], start=True, stop=False)
                    nc.tensor.matmul(f_ps[:], hT[1][:, m * 128:(m + 1) * 128],
                                     w2_sb[1][:], start=False, stop=True)
                    # b2_sb[:, 0:D] holds b2_gamma + 1 (host-folded):
                    # out = nodes + gate*((1+gamma)*nodes + beta)
                    ga = rot.tile([128, D], f32, tag="ga", name="ga")
                    nc.vector.tensor_add(ga[:], f_ps[:, 0:D], b2_sb[:, 0:D])
                    be = rot.tile([128, D], f32, tag="be", name="be")
                    nc.vector.tensor_add(be[:], f_ps[:, D:M2], b2_sb[:, D:M2])
                    nt = nodes_sb[it]
                    nc.vector.tensor_mul(ga[:], ga[:], nt[:])     # (1+gamma)*nodes
                    nc.vector.tensor_add(ga[:], ga[:], be[:])     # + beta
                    nc.vector.tensor_scalar_mul(ga[:], ga[:], gates[m][:])
                    ob = rot.tile([128, D], f32, tag="ob", name="ob")
                    nc.vector.tensor_add(ob[:], ga[:], nt[:])
                    nc.sync.dma_start(out[it * 128:(it + 1) * 128, :], ob[:])

    nc.compile()
    return nc


def kernel(nodes, patient_indices, threshold, temperature, W1, b1, W2, b2):
    from concourse.bass_utils import run_bass_kernel_spmd
    import ml_dtypes

    thresh = float(np.clip(np.asarray(threshold, dtype=np.float64)[0], 0.0, 0.99))
    temp = float(np.asarray(temperature, dtype=np.float64)[0])

    bf = ml_dtypes.bfloat16
    # Sort rows by patient so same-patient pairs live in each core's own
    # diagonal band; the main loop then runs unmasked and a small band
    # correction pass removes the masked pairs. Output rows are unpermuted
    # on the host at the end.
    p_int = np.asarray(patient_indices).astype(np.int64)
    order = np.argsort(p_int, kind="stable")
    nodes = np.ascontiguousarray(np.asarray(nodes, dtype=np.float32)[order])
    p_int = p_int[order]
    xTv = np.ascontiguousarray(nodes.T.astype(bf))                    # [D, B]
    nav = np.empty((B, D + 1), dtype=bf)
    nav[:, 0:D] = nodes.astype(bf)
    nav[:, D] = np.float32(1.0)
    # Relabel patient ids to distinct normal bf16 bit patterns: equality is
    # preserved exactly under f32 compare.
    _, inv = np.unique(p_int, return_inverse=True)
    codes = (np.arange(inv.max() + 1, dtype=np.uint16) + 0x0100).view(bf)
    p_code = codes[inv]  # [B] bf16, distinct value per patient class
    # band (halo) views, zero-padded at the global edges
    xT_pad = np.zeros((D, B + 2 * PAD), dtype=bf)
    xT_pad[:, PAD:PAD + B] = xTv
    na_pad = np.zeros((B + 2 * PAD, D + 1), dtype=bf)
    na_pad[PAD:PAD + B] = nav
    pc_pad = np.zeros(B + 2 * PAD, dtype=np.float32)
    pc_pad[PAD:PAD + B] = p_code.astype(np.float32)
    W1 = np.ascontiguousarray(W1, dtype=np.float32)
    W2 = np.ascontiguousarray(W2, dtype=np.float32)
    b1 = np.asarray(b1, dtype=np.float32)
    b2 = np.asarray(b2, dtype=np.float32)

    p_all = np.ascontiguousarray(p_code.reshape(NJT, 128).T.astype(np.float32))
    b1cv = np.ascontiguousarray(b1.reshape(H // 128, 128).T)          # [128, 2]
    b2x = b2.copy()
    b2x[:D] += 1.0  # fold the FiLM (1+gamma) into the bias broadcast
    b2bv = np.ascontiguousarray(np.broadcast_to(b2x, (128, M2)))      # [128, 512]

    nc = _build(thresh, temp)
    in_maps = []
    for r in range(NCORES):
        sl = slice(r * R, (r + 1) * R)
        b0 = r * R  # band start in padded coords
        in_maps.append({
            "nodes": np.ascontiguousarray(nodes[sl]),
            "xT": xTv,
            "na": nav,
            "xT_band": np.ascontiguousarray(xT_pad[:, b0:b0 + BND]),
            "na_band": np.ascontiguousarray(na_pad[b0:b0 + BND]),
            "pa_band": np.ascontiguousarray(
                pc_pad[b0:b0 + BND].reshape(NBJ, 128).T),
            "p_bcast": np.ascontiguousarray(np.broadcast_to(p_code[sl], (128, R))),
            "p_all": p_all,
            "w1": W1,
            "w2": W2,
            "b1c": b1cv,
            "b2b": b2bv,
        })
    res = run_bass_kernel_spmd(nc, in_maps, list(range(NCORES)),
                               trace=bool(int(__import__("os").environ.get("BASS_KERNEL_TRACE", "0"))))
    kernel.last_results = res
    outp = np.concatenate([res.results[i]["out"] for i in range(NCORES)], axis=0)
    unperm = np.empty_like(outp)
    unperm[order] = outp
    return unperm.astype(np.float32)


kernel.last_results = None
